# revision 27
# baseline (speedup 1.0000x reference)
"""Binary conv1d block (RSign -> BinaryConv1d(k=3, pad=1) -> bias -> RPReLU).

Strategy (8 NeuronCores, data-parallel over batch):
  - Each core gets 2 of the 16 batch images; params replicated.
  - Partition packing p = b*64 + ch puts both images' 64 channels in the
    128 SBUF partitions, so every elementwise op runs at full lane width.
  - RSign: one DVE tensor_scalar is_ge producing g = (x >= alpha) in {1,0}
    as bf16 (exact).  The conv over xb = 2g-1 is recovered affinely:
    y = 2*sum(wb*g) - c + b with per-channel c = sum(wb); the zero-padded
    boundary columns use corrected constants c0/cL.
  - Conv as 3 accumulated 128x128 matmuls per 512-col psum chunk with
    block-diagonal (per-image) +/-1 weights in bf16 -- integer-exact.
  - RPReLU: out = max(beta*y,0) - max(-gamma*y,0) + zeta (beta,gamma >= 0).
    Both relu terms are single ScalarE activations with per-partition
    scale/bias APs reading PSUM; the combine (+zeta fold) is one DVE
    scalar_tensor_tensor.
"""

import os
import sys

import numpy as np

for _p in (
    "/root/.axon_site",
    "/root/.axon_site/_ro/trn_rl_repo",
    "/root/.axon_site/_ro/pypackages",
    "/opt/trn_rl_repo",
    "/opt/pypackages",
):
    if os.path.isdir(_p) and _p not in sys.path:
        sys.path.append(_p)

import ml_dtypes

import concourse.bass as bass
import concourse.tile as tile
from concourse import bacc
from concourse import mybir
from concourse.bass_utils import run_bass_kernel_spmd

B, CIN, COUT, K, L = 16, 64, 64, 3, 65536
NCORES = 8
BPC = B // NCORES  # images per core
P = 128  # partitions = BPC * 64 channels
LT = 2048  # output columns per tile
NT = L // LT
MMN = 512  # matmul free dim (one fp32 PSUM bank)
NCHUNK = LT // MMN

F32 = mybir.dt.float32
BF16 = mybir.dt.bfloat16

LAST_RESULTS = None
_NC_CACHE = {}


def _build_nc(lt=LT, nt=NT, l_total=L, alpha_imm=0.0, zeta_imm=0.0, repeat=1, dt_mult=1, dma_only=False):
    """alpha_imm/zeta_imm: float immediates when those params are
    channel-uniform (walrus rejects TensorScalarPtr with >1 sync wait);
    None selects the general per-partition path.
    repeat: unroll the whole pipeline R times (benchmarking only).
    dt_mult: DMA tile width = dt_mult * lt (compute chunks stay lt wide)."""
    nc = bacc.Bacc()
    x_d = nc.dram_tensor("x", [BPC, CIN, l_total], F32, kind="ExternalInput")
    w_d = nc.dram_tensor("wmats", [P, K * P], BF16, kind="ExternalInput")
    c_d = nc.dram_tensor("consts", [P, 10], F32, kind="ExternalInput")
    o_d = nc.dram_tensor("out", [BPC, COUT, l_total], F32, kind="ExternalOutput")

    x_f = x_d[:].rearrange("b c l -> (b c) l")
    o_f = o_d[:].rearrange("b c l -> (b c) l")

    Relu = mybir.ActivationFunctionType.Relu
    Op = mybir.AluOpType
    nchunk = lt // MMN
    W = lt * dt_mult  # DMA tile width
    assert nt % dt_mult == 0
    nd = nt // dt_mult
    assert nd >= 2, "need at least 2 DMA tiles (separate first/last edges)"
    xin_bufs = 7 if dt_mult == 1 else 3
    out_bufs = 7 if dt_mult == 1 else 3
    g_bufs = 3 if dt_mult == 1 else 2

    # alpha == 0: bf16(x) >= 0 iff x >= 0 (rounding never crosses zero), so
    # the load can cast f32->bf16 in the SDMA datapath, halving the
    # SBUF-port write traffic.  Cast DMAs are SWDGE-only, so loads move to
    # gpsimd and stores take the now-uncontended sync HWDGE ring.
    # Measured on HW: the cast-load saves SBUF-port bytes but not time --
    # the HBM read side (f32 bytes unchanged) is the binder, and SWDGE load
    # dispatch adds overhead.  Keep the HWDGE f32-load path.
    cast_load = False
    xdt = BF16 if cast_load else F32
    load_dma = nc.gpsimd.dma_start if cast_load else nc.sync.dma_start
    store_dma = nc.sync.dma_start if cast_load else nc.gpsimd.dma_start

    with tile.TileContext(nc) as tc:
        with (
            tc.tile_pool(name="const", bufs=1) as constp,
            tc.tile_pool(name="xin", bufs=xin_bufs) as xin,
            tc.tile_pool(name="gbuf", bufs=g_bufs) as gbuf,
            tc.tile_pool(name="am", bufs=4) as am,
            tc.tile_pool(name="outp", bufs=out_bufs) as outp,
            tc.tile_pool(name="fix", bufs=2) as fixp,
            tc.tile_pool(name="ps", bufs=max(1, 4096 // lt), space="PSUM") as psp,
        ):
            wt = constp.tile([P, K * P], BF16)
            nc.sync.dma_start(wt[:], w_d[:])
            ct = constp.tile([P, 10], F32)
            nc.sync.dma_start(ct[:], c_d[:])
            alpha = ct[:, 0:1]
            scA = ct[:, 1:2]
            bA = ct[:, 2:3]
            scM = ct[:, 3:4]
            bM = ct[:, 4:5]
            zeta = ct[:, 5:6]
            bA0 = ct[:, 6:7]
            bM0 = ct[:, 7:8]
            bAL = ct[:, 8:9]
            bML = ct[:, 9:10]

            def binarize(dst, src):
                if alpha_imm is not None:
                    nc.vector.tensor_scalar(dst, src, alpha_imm, None, Op.is_ge)
                else:
                    # general path: consts col 0 holds -alpha; shift on ACT,
                    # then threshold against 0 with a float immediate
                    nc.scalar.add(src, src, alpha)
                    nc.vector.tensor_scalar(dst, src, 0.0, None, Op.is_ge)

            def combine(dst, a, m):
                z = zeta_imm if zeta_imm is not None else zeta
                nc.vector.scalar_tensor_tensor(dst, a, z, m, Op.add, Op.subtract)

            for d in [dd for _ in range(repeat) for dd in range(nd)]:
                D0 = d * W
                x_t = xin.tile([P, W + 2], xdt)
                if dma_only:
                    # bandwidth-floor ablation: load + store only
                    load_dma(x_t[:, 0:W], x_f[:, D0 : D0 + W])
                    store_dma(o_f[:, D0 : D0 + W], x_t[:, 0:W])
                    continue
                g_t = gbuf.tile([P, W + 2], BF16)
                if d == 0:
                    load_dma(x_t[:, 0 : W + 1], x_f[:, 0 : W + 1])
                    nc.vector.memset(g_t[:, 0:1], 0.0)
                    binarize(g_t[:, 1 : W + 2], x_t[:, 0 : W + 1])
                elif d == nd - 1:
                    load_dma(x_t[:, 0 : W + 1], x_f[:, D0 - 1 : D0 + W])
                    nc.vector.memset(g_t[:, W + 1 : W + 2], 0.0)
                    binarize(g_t[:, 0 : W + 1], x_t[:, 0 : W + 1])
                else:
                    load_dma(x_t[:], x_f[:, D0 - 1 : D0 + W + 1])
                    binarize(g_t[:], x_t[:])

                o_t = outp.tile([P, W], F32)
                for ci in range(dt_mult):
                    co = ci * lt
                    ps = psp.tile([P, lt], F32, tag="ps")
                    for j in range(nchunk):
                        for k in range(K):
                            nc.tensor.matmul(
                                ps[:, j * MMN : (j + 1) * MMN],
                                wt[:, k * P : (k + 1) * P],
                                g_t[:, co + j * MMN + k : co + j * MMN + k + MMN],
                                start=(k == 0),
                                stop=(k == K - 1),
                            )

                    a_t = am.tile([P, lt], F32, tag="A")
                    m_t = am.tile([P, lt], F32, tag="M")
                    nc.scalar.activation(a_t[:], ps[:], Relu, bias=bA, scale=scA)
                    nc.scalar.activation(m_t[:], ps[:], Relu, bias=bM, scale=scM)
                    combine(o_t[:, co : co + lt], a_t[:], m_t[:])

                    # boundary columns: missing conv tap -> corrected constants
                    if d == 0 and ci == 0:
                        fa = fixp.tile([P, 1], F32, tag="fa")
                        fm = fixp.tile([P, 1], F32, tag="fm")
                        nc.scalar.activation(
                            fa[:], ps[:, 0:1], Relu, bias=bA0, scale=scA
                        )
                        nc.scalar.activation(
                            fm[:], ps[:, 0:1], Relu, bias=bM0, scale=scM
                        )
                        combine(o_t[:, 0:1], fa[:], fm[:])
                    if d == nd - 1 and ci == dt_mult - 1:
                        fa = fixp.tile([P, 1], F32, tag="fa")
                        fm = fixp.tile([P, 1], F32, tag="fm")
                        nc.scalar.activation(
                            fa[:], ps[:, lt - 1 : lt], Relu, bias=bAL, scale=scA
                        )
                        nc.scalar.activation(
                            fm[:], ps[:, lt - 1 : lt], Relu, bias=bML, scale=scM
                        )
                        combine(o_t[:, W - 1 : W], fa[:], fm[:])

                store_dma(o_f[:, D0 : D0 + W], o_t[:])
    nc.compile()
    return nc


def _prep_params(w, b, alpha, beta, gamma, zeta):
    w = np.asarray(w, np.float32)
    b = np.asarray(b, np.float32).reshape(COUT)
    al = np.asarray(alpha, np.float32).reshape(CIN)
    be = np.asarray(beta, np.float32).reshape(COUT)
    ga = np.asarray(gamma, np.float32).reshape(COUT)
    ze = np.asarray(zeta, np.float32).reshape(COUT)
    assert (be >= 0).all() and (ga >= 0).all(), (
        "kernel assumes beta, gamma >= 0 (RPReLU slopes)"
    )

    wb = np.where(w >= 0, np.float32(1.0), np.float32(-1.0))  # [COUT, CIN, K]
    c = wb.sum(axis=(1, 2), dtype=np.float32)  # interior correction
    c0 = c - wb[:, :, 0].sum(axis=1, dtype=np.float32)  # l = 0 (no left tap)
    cL = c - wb[:, :, K - 1].sum(axis=1, dtype=np.float32)  # l = L-1

    # block-diagonal lhsT per tap: rows (img, cin) -> cols (img, cout)
    wm = np.zeros((P, K * P), np.float32)
    for k in range(K):
        blk = wb[:, :, k].T  # [CIN, COUT]
        for i in range(BPC):
            wm[
                i * CIN : (i + 1) * CIN, k * P + i * COUT : k * P + (i + 1) * COUT
            ] = blk
    wm = wm.astype(ml_dtypes.bfloat16)

    def t2(v):
        return np.tile(np.asarray(v, np.float32), BPC)[:, None]

    consts = np.concatenate(
        [
            t2(-al),
            t2(2.0 * be),
            t2(be * (b - c)),
            t2(-2.0 * ga),
            t2(ga * (c - b)),
            t2(ze),
            t2(be * (b - c0)),
            t2(ga * (c0 - b)),
            t2(be * (b - cL)),
            t2(ga * (cL - b)),
        ],
        axis=1,
    ).astype(np.float32)
    return wm, consts


def kernel(x, w, b, alpha, beta, gamma, zeta):
    global LAST_RESULTS
    x = np.ascontiguousarray(np.asarray(x), dtype=np.float32)
    assert x.shape == (B, CIN, L)
    wm, consts = _prep_params(w, b, alpha, beta, gamma, zeta)

    al = np.asarray(alpha, np.float32).ravel()
    ze = np.asarray(zeta, np.float32).ravel()
    alpha_imm = float(al[0]) if np.all(al == al[0]) else None
    zeta_imm = float(ze[0]) if np.all(ze == ze[0]) else None

    key = ("nc", alpha_imm is None, zeta_imm is None, alpha_imm, zeta_imm)
    if key not in _NC_CACHE:
        _NC_CACHE[key] = _build_nc(alpha_imm=alpha_imm, zeta_imm=zeta_imm)
    nc = _NC_CACHE[key]

    in_maps = [
        {"x": x[i * BPC : (i + 1) * BPC], "wmats": wm, "consts": consts}
        for i in range(NCORES)
    ]
    res = run_bass_kernel_spmd(
        nc,
        in_maps,
        list(range(NCORES)),
        trace=bool(int(os.environ.get("KERNEL_TRACE", "0"))),
    )
    LAST_RESULTS = res
    out = np.concatenate([res.results[i]["out"] for i in range(NCORES)], axis=0)
    return out
